# revision 5
# baseline (speedup 1.0000x reference)
"""Trainium2 Bass kernel: segment_sum of edge features into nodes (GNN aggregation).

Computes out[n, :] = sum over edges e with receivers[e] == n of edges[e, :],
for n in [0, 100000), edges [1000000, 64] fp32 — distributed over 8 NeuronCores
(value-sharded by receiver range, 12500 nodes per core; node ranges are
disjoint so no cross-core reduction is needed).

Device algorithm ("degree-slotted static-ones matmul"):
  - Host packs each node's edges (sorted by receiver) into "node-rows" of 4
    slots; a chunk = 32 node-rows = 128 slots = one TensorEngine matmul.
  - The stationary operand is a single STATIC block-ones matrix [128, 32]
    (ones[s, j] = 1 iff s//4 == j): out[j, :] = sum of the 4 slots of row j.
    No per-chunk weight generation at all (no one-hot, zero VectorEngine work).
  - Edge fp32 values ride as fp16 hi + fp16 lo halves in one matmul: the
    output access pattern wraps both 64-column halves onto the same PSUM
    columns and PSUM's per-element accumulate folds hi+lo (error ~1e-7).
  - Four chunks pack one 128-partition PSUM block via column tiling
    (tile_position=(0, 32b)); 7 blocks share a PSUM bank; one ScalarEngine
    copy flushes the bank, then a contiguous DMA writes the rows out.
  - Host folds the ~3 rows per node with one np.add.reduceat.
"""

import os

import numpy as np

N_EDGES = 1_000_000
N_NODES = 100_000
N_FEAT = 64
N_CORES = 8
NODES_PER_CORE = N_NODES // N_CORES  # 12500
S = 4  # slots per node-row
BLK_W = 7  # 128-row blocks per PSUM bank / flush group
N_BLOCKS = 294  # row capacity 294*128 = 37632 (mean demand ~36625)
ROWS_CAP = N_BLOCKS * 128
C_CHUNKS = N_BLOCKS * 4  # 1176 chunks of 128 slots
N_GROUPS = N_BLOCKS // BLK_W  # 42

_NC_CACHE = None
LAST_RESULT = None  # BassKernelResults of the most recent hardware run


def _build_nc():
    global _NC_CACHE
    if _NC_CACHE is not None:
        return _NC_CACHE

    import concourse.bass as bass
    import concourse.tile as tile
    from concourse import bacc, mybir

    F16 = mybir.dt.float16
    F32 = mybir.dt.float32

    nc = bacc.Bacc("TRN2", target_bir_lowering=False)
    tokens = nc.dram_tensor("tokens", [128, C_CHUNKS, 128], F16, kind="ExternalInput")
    ones = nc.dram_tensor("ones", [128, 32], F16, kind="ExternalInput")
    out = nc.dram_tensor("out", [128, N_BLOCKS, 64], F32, kind="ExternalOutput")

    with tile.TileContext(nc) as tc:
        with (
            tc.tile_pool(name="const", bufs=1) as const,
            tc.tile_pool(name="tok", bufs=5) as tokp,
            tc.tile_pool(name="ps", bufs=4, space="PSUM") as psp,
            tc.tile_pool(name="stage", bufs=3) as stp,
        ):
            ones_t = const.tile([128, 32], F16)
            nc.sync.dma_start(ones_t[:], ones[:])

            for g in range(N_GROUPS):
                c0 = g * BLK_W * 4
                tok = tokp.tile([128, BLK_W * 4, 128], F16)
                half = BLK_W * 2
                nc.sync.dma_start(tok[:, 0:half, :], tokens[:, c0 : c0 + half, :])
                nc.sync.dma_start(
                    tok[:, half : BLK_W * 4, :],
                    tokens[:, c0 + half : c0 + BLK_W * 4, :])
                ps = psp.tile([128, BLK_W * 64], F32)
                for blk in range(BLK_W):
                    for b in range(4):
                        lc = blk * 4 + b
                        pslice = ps[32 * b : 32 * b + 32,
                                    blk * 64 : (blk + 1) * 64]
                        o = bass.AP(pslice.tensor, pslice.offset,
                                    [list(pslice.ap[0]), [0, 2], [1, 64]])
                        nc.tensor.matmul(
                            out=o, lhsT=ones_t[:], rhs=tok[:, lc, :],
                            start=True, stop=True, tile_position=(0, 32 * b))
                stage = stp.tile([128, BLK_W * 64], F32)
                nc.scalar.copy(stage[:], ps[:])
                nc.sync.dma_start(out[:, g * BLK_W : (g + 1) * BLK_W, :], stage[:])
    nc.compile()
    _NC_CACHE = nc
    return nc


def _numpy_segment_sum(edges, receivers):
    out = np.zeros((N_NODES, N_FEAT), np.float32)
    r = np.asarray(receivers).astype(np.int64)
    ok = (r >= 0) & (r < N_NODES)
    np.add.at(out, r[ok], np.asarray(edges, np.float32)[ok])
    return out


def kernel(edges, nodes, receivers):
    global LAST_RESULT

    edges = np.ascontiguousarray(edges, dtype=np.float32)
    n_nodes = nodes.shape[0]
    r = np.asarray(receivers).astype(np.int64)
    if (
        edges.shape != (N_EDGES, N_FEAT)
        or n_nodes != N_NODES
        or r.shape != (N_EDGES,)
        or os.environ.get("KERNEL_FORCE_NUMPY")
    ):
        return _numpy_segment_sum(edges, receivers)

    # ---- host-side sharding / packing ----
    order = np.argsort(r, kind="stable")
    r_s = r[order]
    bounds = np.searchsorted(r_s, NODES_PER_CORE * np.arange(N_CORES + 1))

    hi_all = edges.astype(np.float16)
    lo_all = (edges - hi_all.astype(np.float32)).astype(np.float16)

    ones = np.zeros((128, 32), np.float16)
    ones[np.arange(128), np.arange(128) // S] = 1.0

    in_maps = []
    spill_ids = []
    meta = []
    for i in range(N_CORES):
        lo_b, hi_b = bounds[i], bounds[i + 1]
        idx = order[lo_b:hi_b]
        rr = r_s[lo_b:hi_b] - NODES_PER_CORE * i

        d = np.bincount(rr, minlength=NODES_PER_CORE)
        rows_n = (d + S - 1) // S
        total = rows_n.sum()
        if total > ROWS_CAP:
            # Spill whole tail nodes to a host-side fixup (pathological skew).
            cut = int(np.searchsorted(np.cumsum(rows_n), ROWS_CAP, side="right"))
            sp = rr >= cut
            spill_ids.append(idx[sp])
            idx, rr = idx[~sp], rr[~sp]
            d = np.bincount(rr, minlength=NODES_PER_CORE)
            rows_n = (d + S - 1) // S

        rowstart_n = np.zeros(NODES_PER_CORE, np.int64)
        np.cumsum(rows_n[:-1], out=rowstart_n[1:])
        node_first = np.zeros(NODES_PER_CORE, np.int64)
        np.cumsum(d[:-1], out=node_first[1:])

        rank = np.arange(len(rr)) - node_first[rr]
        row_e = rowstart_n[rr] + (rank >> 2)
        slot_e = rank & (S - 1)

        tokens = np.zeros((128, C_CHUNKS, 128), np.float16)
        lc = row_e >> 5
        p = (row_e & 31) * S + slot_e
        tokens[p, lc, 0:64] = hi_all[idx]
        tokens[p, lc, 64:128] = lo_all[idx]
        in_maps.append({"tokens": tokens, "ones": ones})
        meta.append((rows_n, rowstart_n))

    # ---- device run ----
    from concourse.bass_utils import run_bass_kernel_spmd

    nc = _build_nc()
    res = run_bass_kernel_spmd(nc, in_maps, core_ids=list(range(N_CORES)))
    LAST_RESULT = res

    # ---- unshard: fold node-rows back into nodes ----
    full = np.zeros((N_NODES, N_FEAT), np.float32)
    for i in range(N_CORES):
        dev = res.results[i]["out"]  # [128, N_BLOCKS, 64]
        rows_arr = dev.transpose(1, 0, 2).reshape(ROWS_CAP, 64)
        rows_n, rowstart_n = meta[i]
        nz = rows_n > 0
        if nz.any():
            seg = np.add.reduceat(rows_arr, rowstart_n[nz])
            block = full[i * NODES_PER_CORE : (i + 1) * NODES_PER_CORE]
            block[nz] = seg

    if spill_ids:
        sp = np.concatenate(spill_ids)
        np.add.at(full, r[sp], edges[sp])

    return full


# revision 6
# speedup vs baseline: 1.1053x; 1.1053x over previous
"""Trainium2 Bass kernel: segment_sum of edge features into nodes (GNN aggregation).

Computes out[n, :] = sum over edges e with receivers[e] == n of edges[e, :],
for n in [0, 100000), edges [1000000, 64] fp32 — distributed over 8 NeuronCores
(value-sharded by receiver range, 12500 nodes per core; node ranges are
disjoint so no cross-core reduction is needed).

Device algorithm ("degree-slotted static-ones matmul"):
  - Host packs each node's edges (sorted by receiver) into "node-rows" of 4
    slots; a chunk = 32 node-rows = 128 slots = one TensorEngine matmul.
  - The stationary operand is a single STATIC block-ones matrix [128, 32]
    (ones[s, j] = 1 iff s//4 == j): out[j, :] = sum of the 4 slots of row j.
    No per-chunk weight generation at all (no one-hot, zero VectorEngine work).
  - Edge fp32 values ride as fp16 hi + fp16 lo halves in one matmul: the
    output access pattern wraps both 64-column halves onto the same PSUM
    columns and PSUM's per-element accumulate folds hi+lo (error ~1e-7).
  - Four chunks pack one 128-partition PSUM block via column tiling
    (tile_position=(0, 32b)); 7 blocks share a PSUM bank; one ScalarEngine
    copy flushes the bank, then a contiguous DMA writes the rows out.
  - Host folds the ~3 rows per node with one np.add.reduceat.
"""

import os

import numpy as np

N_EDGES = 1_000_000
N_NODES = 100_000
N_FEAT = 64
N_CORES = 8
NODES_PER_CORE = N_NODES // N_CORES  # 12500
S = 4  # slots per node-row
BLK_W = 7  # 128-row blocks per PSUM bank / flush group
N_BLOCKS = 294  # row capacity 294*128 = 37632 (mean demand ~36625)
ROWS_CAP = N_BLOCKS * 128
C_CHUNKS = N_BLOCKS * 4  # 1176 chunks of 128 slots
N_GROUPS = N_BLOCKS // BLK_W  # 42

_NC_CACHE = None
LAST_RESULT = None  # BassKernelResults of the most recent hardware run


def _build_nc():
    global _NC_CACHE
    if _NC_CACHE is not None:
        return _NC_CACHE

    import concourse.bass as bass
    import concourse.tile as tile
    from concourse import bacc, mybir

    F16 = mybir.dt.float16
    F32 = mybir.dt.float32

    nc = bacc.Bacc("TRN2", target_bir_lowering=False)
    tokens = nc.dram_tensor("tokens", [128, C_CHUNKS, 128], F16, kind="ExternalInput")
    ones = nc.dram_tensor("ones", [128, 32], F16, kind="ExternalInput")
    out = nc.dram_tensor("out", [128, N_BLOCKS, 64], F32, kind="ExternalOutput")

    with tile.TileContext(nc) as tc:
        with (
            tc.tile_pool(name="const", bufs=1) as const,
            tc.tile_pool(name="tok", bufs=4) as tokp,
            tc.tile_pool(name="ps", bufs=2, space="PSUM") as psp,
            tc.tile_pool(name="stage", bufs=2) as stp,
        ):
            ones_t = const.tile([128, 32], F16)
            nc.sync.dma_start(ones_t[:], ones[:])

            for g in range(N_GROUPS):
                c0 = g * BLK_W * 4
                tok = tokp.tile([128, BLK_W * 4, 128], F16)
                nc.sync.dma_start(tok[:], tokens[:, c0 : c0 + BLK_W * 4, :])
                ps = psp.tile([128, BLK_W * 64], F32)
                for blk in range(BLK_W):
                    for b in range(4):
                        lc = blk * 4 + b
                        pslice = ps[32 * b : 32 * b + 32,
                                    blk * 64 : (blk + 1) * 64]
                        o = bass.AP(pslice.tensor, pslice.offset,
                                    [list(pslice.ap[0]), [0, 2], [1, 64]])
                        nc.tensor.matmul(
                            out=o, lhsT=ones_t[:], rhs=tok[:, lc, :],
                            start=True, stop=True, tile_position=(0, 32 * b))
                stage = stp.tile([128, BLK_W * 64], F32)
                nc.scalar.copy(stage[:], ps[:])
                nc.sync.dma_start(out[:, g * BLK_W : (g + 1) * BLK_W, :], stage[:])
    nc.compile()
    _NC_CACHE = nc
    return nc


def _numpy_segment_sum(edges, receivers):
    out = np.zeros((N_NODES, N_FEAT), np.float32)
    r = np.asarray(receivers).astype(np.int64)
    ok = (r >= 0) & (r < N_NODES)
    np.add.at(out, r[ok], np.asarray(edges, np.float32)[ok])
    return out


def kernel(edges, nodes, receivers):
    global LAST_RESULT

    edges = np.ascontiguousarray(edges, dtype=np.float32)
    n_nodes = nodes.shape[0]
    r = np.asarray(receivers).astype(np.int64)
    if (
        edges.shape != (N_EDGES, N_FEAT)
        or n_nodes != N_NODES
        or r.shape != (N_EDGES,)
        or os.environ.get("KERNEL_FORCE_NUMPY")
    ):
        return _numpy_segment_sum(edges, receivers)

    # ---- host-side sharding / packing ----
    order = np.argsort(r, kind="stable")
    r_s = r[order]
    bounds = np.searchsorted(r_s, NODES_PER_CORE * np.arange(N_CORES + 1))

    hi_all = edges.astype(np.float16)
    lo_all = (edges - hi_all.astype(np.float32)).astype(np.float16)

    ones = np.zeros((128, 32), np.float16)
    ones[np.arange(128), np.arange(128) // S] = 1.0

    in_maps = []
    spill_ids = []
    meta = []
    for i in range(N_CORES):
        lo_b, hi_b = bounds[i], bounds[i + 1]
        idx = order[lo_b:hi_b]
        rr = r_s[lo_b:hi_b] - NODES_PER_CORE * i

        d = np.bincount(rr, minlength=NODES_PER_CORE)
        rows_n = (d + S - 1) // S
        total = rows_n.sum()
        if total > ROWS_CAP:
            # Spill whole tail nodes to a host-side fixup (pathological skew).
            cut = int(np.searchsorted(np.cumsum(rows_n), ROWS_CAP, side="right"))
            sp = rr >= cut
            spill_ids.append(idx[sp])
            idx, rr = idx[~sp], rr[~sp]
            d = np.bincount(rr, minlength=NODES_PER_CORE)
            rows_n = (d + S - 1) // S

        rowstart_n = np.zeros(NODES_PER_CORE, np.int64)
        np.cumsum(rows_n[:-1], out=rowstart_n[1:])
        node_first = np.zeros(NODES_PER_CORE, np.int64)
        np.cumsum(d[:-1], out=node_first[1:])

        rank = np.arange(len(rr)) - node_first[rr]
        row_e = rowstart_n[rr] + (rank >> 2)
        slot_e = rank & (S - 1)

        tokens = np.zeros((128, C_CHUNKS, 128), np.float16)
        lc = row_e >> 5
        p = (row_e & 31) * S + slot_e
        tokens[p, lc, 0:64] = hi_all[idx]
        tokens[p, lc, 64:128] = lo_all[idx]
        in_maps.append({"tokens": tokens, "ones": ones})
        meta.append((rows_n, rowstart_n))

    # ---- device run ----
    from concourse.bass_utils import run_bass_kernel_spmd

    nc = _build_nc()
    res = run_bass_kernel_spmd(nc, in_maps, core_ids=list(range(N_CORES)))
    LAST_RESULT = res

    # ---- unshard: fold node-rows back into nodes ----
    full = np.zeros((N_NODES, N_FEAT), np.float32)
    for i in range(N_CORES):
        dev = res.results[i]["out"]  # [128, N_BLOCKS, 64]
        rows_arr = dev.transpose(1, 0, 2).reshape(ROWS_CAP, 64)
        rows_n, rowstart_n = meta[i]
        nz = rows_n > 0
        if nz.any():
            seg = np.add.reduceat(rows_arr, rowstart_n[nz])
            block = full[i * NODES_PER_CORE : (i + 1) * NODES_PER_CORE]
            block[nz] = seg

    if spill_ids:
        sp = np.concatenate(spill_ids)
        np.add.at(full, r[sp], edges[sp])

    return full


# revision 7
# speedup vs baseline: 1.1289x; 1.0214x over previous
"""Trainium2 Bass kernel: segment_sum of edge features into nodes (GNN aggregation).

Computes out[n, :] = sum over edges e with receivers[e] == n of edges[e, :],
for n in [0, 100000), edges [1000000, 64] fp32 — distributed over 8 NeuronCores
(value-sharded by receiver range, 12500 nodes per core; node ranges are
disjoint so no cross-core reduction is needed).

Device algorithm ("degree-slotted static-ones matmul"):
  - Host packs each node's edges (sorted by receiver) into "node-rows" of 4
    slots; a chunk = 32 node-rows = 128 slots = one TensorEngine matmul.
  - The stationary operand is a single STATIC block-ones matrix [128, 32]
    (ones[s, j] = 1 iff s//4 == j): out[j, :] = sum of the 4 slots of row j.
    No per-chunk weight generation at all (no one-hot, zero VectorEngine work).
  - Edge fp32 values ride as fp16 hi + fp16 lo halves in one matmul: the
    output access pattern wraps both 64-column halves onto the same PSUM
    columns and PSUM's per-element accumulate folds hi+lo (error ~1e-7).
  - Four chunks pack one 128-partition PSUM block via column tiling
    (tile_position=(0, 32b)); 7 blocks share a PSUM bank; one ScalarEngine
    copy flushes the bank, then a contiguous DMA writes the rows out.
  - Host folds the ~3 rows per node with one np.add.reduceat.
"""

import os

import numpy as np

N_EDGES = 1_000_000
N_NODES = 100_000
N_FEAT = 64
N_CORES = 8
NODES_PER_CORE = N_NODES // N_CORES  # 12500
S = 4  # slots per node-row
BLK_W = 7  # 128-row blocks per PSUM bank / flush group
N_BLOCKS = 294  # row capacity 294*128 = 37632 (mean demand ~36625)
ROWS_CAP = N_BLOCKS * 128
C_CHUNKS = N_BLOCKS * 4  # 1176 chunks of 128 slots
N_GROUPS = N_BLOCKS // BLK_W  # 42

_NC_CACHE = None
LAST_RESULT = None  # BassKernelResults of the most recent hardware run


def _build_nc():
    global _NC_CACHE
    if _NC_CACHE is not None:
        return _NC_CACHE

    import concourse.bass as bass
    import concourse.tile as tile
    from concourse import bacc, mybir

    F16 = mybir.dt.float16
    F32 = mybir.dt.float32

    nc = bacc.Bacc("TRN2", target_bir_lowering=False)
    tokens = nc.dram_tensor("tokens", [128, C_CHUNKS, 128], F16, kind="ExternalInput")
    ones = nc.dram_tensor("ones", [128, 32], F16, kind="ExternalInput")
    out = nc.dram_tensor("out", [128, N_BLOCKS, 64], F32, kind="ExternalOutput")

    with tile.TileContext(nc) as tc:
        with (
            tc.tile_pool(name="const", bufs=1) as const,
            tc.tile_pool(name="tok", bufs=4) as tokp,
            tc.tile_pool(name="ps", bufs=2, space="PSUM") as psp,
            tc.tile_pool(name="stage", bufs=2) as stp,
        ):
            ones_t = const.tile([128, 32], F16)
            nc.sync.dma_start(ones_t[:], ones[:])

            # progressive group sizes: small first groups hide the DMA ramp
            group_sizes = [1, 1, 2, 3] + [BLK_W] * ((N_BLOCKS - 7) // BLK_W)
            assert sum(group_sizes) == N_BLOCKS
            b0 = 0
            for gw in group_sizes:
                c0 = b0 * 4
                tok = tokp.tile([128, BLK_W * 4, 128], F16, tag="tok")
                nc.sync.dma_start(
                    tok[:, 0 : gw * 4, :], tokens[:, c0 : c0 + gw * 4, :])
                ps = psp.tile([128, BLK_W * 64], F32, tag="ps")
                for blk in range(gw):
                    for b in range(4):
                        lc = blk * 4 + b
                        pslice = ps[32 * b : 32 * b + 32,
                                    blk * 64 : (blk + 1) * 64]
                        o = bass.AP(pslice.tensor, pslice.offset,
                                    [list(pslice.ap[0]), [0, 2], [1, 64]])
                        nc.tensor.matmul(
                            out=o, lhsT=ones_t[:], rhs=tok[:, lc, :],
                            start=True, stop=True, tile_position=(0, 32 * b))
                stage = stp.tile([128, BLK_W * 64], F32, tag="stage")
                nc.scalar.copy(stage[:, 0 : gw * 64], ps[:, 0 : gw * 64])
                nc.sync.dma_start(
                    out[:, b0 : b0 + gw, :], stage[:, 0 : gw * 64])
                b0 += gw
    nc.compile()
    _NC_CACHE = nc
    return nc


def _numpy_segment_sum(edges, receivers):
    out = np.zeros((N_NODES, N_FEAT), np.float32)
    r = np.asarray(receivers).astype(np.int64)
    ok = (r >= 0) & (r < N_NODES)
    np.add.at(out, r[ok], np.asarray(edges, np.float32)[ok])
    return out


def kernel(edges, nodes, receivers):
    global LAST_RESULT

    edges = np.ascontiguousarray(edges, dtype=np.float32)
    n_nodes = nodes.shape[0]
    r = np.asarray(receivers).astype(np.int64)
    if (
        edges.shape != (N_EDGES, N_FEAT)
        or n_nodes != N_NODES
        or r.shape != (N_EDGES,)
        or os.environ.get("KERNEL_FORCE_NUMPY")
    ):
        return _numpy_segment_sum(edges, receivers)

    # ---- host-side sharding / packing ----
    order = np.argsort(r, kind="stable")
    r_s = r[order]
    bounds = np.searchsorted(r_s, NODES_PER_CORE * np.arange(N_CORES + 1))

    hi_all = edges.astype(np.float16)
    lo_all = (edges - hi_all.astype(np.float32)).astype(np.float16)

    ones = np.zeros((128, 32), np.float16)
    ones[np.arange(128), np.arange(128) // S] = 1.0

    in_maps = []
    spill_ids = []
    meta = []
    for i in range(N_CORES):
        lo_b, hi_b = bounds[i], bounds[i + 1]
        idx = order[lo_b:hi_b]
        rr = r_s[lo_b:hi_b] - NODES_PER_CORE * i

        d = np.bincount(rr, minlength=NODES_PER_CORE)
        rows_n = (d + S - 1) // S
        total = rows_n.sum()
        if total > ROWS_CAP:
            # Spill whole tail nodes to a host-side fixup (pathological skew).
            cut = int(np.searchsorted(np.cumsum(rows_n), ROWS_CAP, side="right"))
            sp = rr >= cut
            spill_ids.append(idx[sp])
            idx, rr = idx[~sp], rr[~sp]
            d = np.bincount(rr, minlength=NODES_PER_CORE)
            rows_n = (d + S - 1) // S

        rowstart_n = np.zeros(NODES_PER_CORE, np.int64)
        np.cumsum(rows_n[:-1], out=rowstart_n[1:])
        node_first = np.zeros(NODES_PER_CORE, np.int64)
        np.cumsum(d[:-1], out=node_first[1:])

        rank = np.arange(len(rr)) - node_first[rr]
        row_e = rowstart_n[rr] + (rank >> 2)
        slot_e = rank & (S - 1)

        tokens = np.zeros((128, C_CHUNKS, 128), np.float16)
        lc = row_e >> 5
        p = (row_e & 31) * S + slot_e
        tokens[p, lc, 0:64] = hi_all[idx]
        tokens[p, lc, 64:128] = lo_all[idx]
        in_maps.append({"tokens": tokens, "ones": ones})
        meta.append((rows_n, rowstart_n))

    # ---- device run ----
    from concourse.bass_utils import run_bass_kernel_spmd

    nc = _build_nc()
    res = run_bass_kernel_spmd(nc, in_maps, core_ids=list(range(N_CORES)))
    LAST_RESULT = res

    # ---- unshard: fold node-rows back into nodes ----
    full = np.zeros((N_NODES, N_FEAT), np.float32)
    for i in range(N_CORES):
        dev = res.results[i]["out"]  # [128, N_BLOCKS, 64]
        rows_arr = dev.transpose(1, 0, 2).reshape(ROWS_CAP, 64)
        rows_n, rowstart_n = meta[i]
        nz = rows_n > 0
        if nz.any():
            seg = np.add.reduceat(rows_arr, rowstart_n[nz])
            block = full[i * NODES_PER_CORE : (i + 1) * NODES_PER_CORE]
            block[nz] = seg

    if spill_ids:
        sp = np.concatenate(spill_ids)
        np.add.at(full, r[sp], edges[sp])

    return full


# revision 8
# speedup vs baseline: 1.2477x; 1.1052x over previous
"""Trainium2 Bass kernel: segment_sum of edge features into nodes (GNN aggregation).

Computes out[n, :] = sum over edges e with receivers[e] == n of edges[e, :],
for n in [0, 100000), edges [1000000, 64] fp32 — distributed over 8 NeuronCores
(value-sharded by receiver range, 12500 nodes per core; node ranges are
disjoint so no cross-core reduction is needed).

Device algorithm ("degree-slotted static-ones matmul"):
  - Host packs each node's edges (sorted by receiver) into "node-rows" of 4
    slots; a chunk = 32 node-rows = 128 slots = one TensorEngine matmul.
  - The stationary operand is a single STATIC block-ones matrix [128, 32]
    (ones[s, j] = 1 iff s//4 == j): out[j, :] = sum of the 4 slots of row j.
    No per-chunk weight generation at all (no one-hot, zero VectorEngine work).
  - Edge fp32 values ride as fp16 hi + fp16 lo halves in one matmul: the
    output access pattern wraps both 64-column halves onto the same PSUM
    columns and PSUM's per-element accumulate folds hi+lo (error ~1e-7).
  - Four chunks pack one 128-partition PSUM block via column tiling
    (tile_position=(0, 32b)); 7 blocks share a PSUM bank; one ScalarEngine
    copy flushes the bank, then a contiguous DMA writes the rows out.
  - Host folds the ~3 rows per node with one np.add.reduceat.
"""

import os

import numpy as np

N_EDGES = 1_000_000
N_NODES = 100_000
N_FEAT = 64
N_CORES = 8
NODES_PER_CORE = N_NODES // N_CORES  # 12500
S = 4  # slots per node-row
BLK_W = 7  # 128-row blocks per PSUM bank / flush group
N_BLOCKS = 294  # row capacity 294*128 = 37632 (mean demand ~36625)
ROWS_CAP = N_BLOCKS * 128
C_CHUNKS = N_BLOCKS * 4  # 1176 chunks of 128 slots
N_GROUPS = N_BLOCKS // BLK_W  # 42

_NC_CACHE = None
LAST_RESULT = None  # BassKernelResults of the most recent hardware run


def _build_nc():
    global _NC_CACHE
    if _NC_CACHE is not None:
        return _NC_CACHE

    import concourse.bass as bass
    import concourse.tile as tile
    from concourse import bacc, mybir

    F16 = mybir.dt.float16
    F32 = mybir.dt.float32

    nc = bacc.Bacc("TRN2", target_bir_lowering=False)
    tokens = nc.dram_tensor("tokens", [128, C_CHUNKS, 128], F16, kind="ExternalInput")
    ones = nc.dram_tensor("ones", [128, 32], F16, kind="ExternalInput")
    out = nc.dram_tensor("out", [128, N_BLOCKS, 64], F32, kind="ExternalOutput")

    with tile.TileContext(nc) as tc:
        with (
            tc.tile_pool(name="const", bufs=1) as const,
            tc.tile_pool(name="tok", bufs=4) as tokp,
            tc.tile_pool(name="ps", bufs=2, space="PSUM") as psp,
            tc.tile_pool(name="stage", bufs=2) as stp,
        ):
            ones_t = const.tile([128, 32], F16)
            nc.sync.dma_start(ones_t[:], ones[:])

            # progressive group sizes: small first groups hide the DMA ramp
            group_sizes = [1, 1, 2, 3] + [BLK_W] * ((N_BLOCKS - 7) // BLK_W)
            assert sum(group_sizes) == N_BLOCKS
            b0 = 0
            for gw in group_sizes:
                c0 = b0 * 4
                tok = tokp.tile([128, BLK_W * 4, 128], F16, tag="tok")
                nc.sync.dma_start(
                    tok[:, 0 : gw * 4, :], tokens[:, c0 : c0 + gw * 4, :])
                ps = psp.tile([128, BLK_W * 64], F32, tag="ps")
                for blk in range(gw):
                    for b in range(4):
                        lc = blk * 4 + b
                        pslice = ps[32 * b : 32 * b + 32,
                                    blk * 64 : (blk + 1) * 64]
                        o = bass.AP(pslice.tensor, pslice.offset,
                                    [list(pslice.ap[0]), [0, 2], [1, 64]])
                        nc.tensor.matmul(
                            out=o, lhsT=ones_t[:], rhs=tok[:, lc, :],
                            start=True, stop=True, tile_position=(0, 32 * b))
                stage = stp.tile([128, BLK_W * 64], F32, tag="stage")
                nc.scalar.copy(stage[:, 0 : gw * 64], ps[:, 0 : gw * 64])
                nc.scalar.dma_start(
                    out[:, b0 : b0 + gw, :], stage[:, 0 : gw * 64])
                b0 += gw
    nc.compile()
    _NC_CACHE = nc
    return nc


def _numpy_segment_sum(edges, receivers):
    out = np.zeros((N_NODES, N_FEAT), np.float32)
    r = np.asarray(receivers).astype(np.int64)
    ok = (r >= 0) & (r < N_NODES)
    np.add.at(out, r[ok], np.asarray(edges, np.float32)[ok])
    return out


def kernel(edges, nodes, receivers):
    global LAST_RESULT

    edges = np.ascontiguousarray(edges, dtype=np.float32)
    n_nodes = nodes.shape[0]
    r = np.asarray(receivers).astype(np.int64)
    if (
        edges.shape != (N_EDGES, N_FEAT)
        or n_nodes != N_NODES
        or r.shape != (N_EDGES,)
        or os.environ.get("KERNEL_FORCE_NUMPY")
    ):
        return _numpy_segment_sum(edges, receivers)

    # ---- host-side sharding / packing ----
    order = np.argsort(r, kind="stable")
    r_s = r[order]
    bounds = np.searchsorted(r_s, NODES_PER_CORE * np.arange(N_CORES + 1))

    hi_all = edges.astype(np.float16)
    lo_all = (edges - hi_all.astype(np.float32)).astype(np.float16)

    ones = np.zeros((128, 32), np.float16)
    ones[np.arange(128), np.arange(128) // S] = 1.0

    in_maps = []
    spill_ids = []
    meta = []
    for i in range(N_CORES):
        lo_b, hi_b = bounds[i], bounds[i + 1]
        idx = order[lo_b:hi_b]
        rr = r_s[lo_b:hi_b] - NODES_PER_CORE * i

        d = np.bincount(rr, minlength=NODES_PER_CORE)
        rows_n = (d + S - 1) // S
        total = rows_n.sum()
        if total > ROWS_CAP:
            # Spill whole tail nodes to a host-side fixup (pathological skew).
            cut = int(np.searchsorted(np.cumsum(rows_n), ROWS_CAP, side="right"))
            sp = rr >= cut
            spill_ids.append(idx[sp])
            idx, rr = idx[~sp], rr[~sp]
            d = np.bincount(rr, minlength=NODES_PER_CORE)
            rows_n = (d + S - 1) // S

        rowstart_n = np.zeros(NODES_PER_CORE, np.int64)
        np.cumsum(rows_n[:-1], out=rowstart_n[1:])
        node_first = np.zeros(NODES_PER_CORE, np.int64)
        np.cumsum(d[:-1], out=node_first[1:])

        rank = np.arange(len(rr)) - node_first[rr]
        row_e = rowstart_n[rr] + (rank >> 2)
        slot_e = rank & (S - 1)

        tokens = np.zeros((128, C_CHUNKS, 128), np.float16)
        lc = row_e >> 5
        p = (row_e & 31) * S + slot_e
        tokens[p, lc, 0:64] = hi_all[idx]
        tokens[p, lc, 64:128] = lo_all[idx]
        in_maps.append({"tokens": tokens, "ones": ones})
        meta.append((rows_n, rowstart_n))

    # ---- device run ----
    from concourse.bass_utils import run_bass_kernel_spmd

    nc = _build_nc()
    res = run_bass_kernel_spmd(nc, in_maps, core_ids=list(range(N_CORES)))
    LAST_RESULT = res

    # ---- unshard: fold node-rows back into nodes ----
    full = np.zeros((N_NODES, N_FEAT), np.float32)
    for i in range(N_CORES):
        dev = res.results[i]["out"]  # [128, N_BLOCKS, 64]
        rows_arr = dev.transpose(1, 0, 2).reshape(ROWS_CAP, 64)
        rows_n, rowstart_n = meta[i]
        nz = rows_n > 0
        if nz.any():
            seg = np.add.reduceat(rows_arr, rowstart_n[nz])
            block = full[i * NODES_PER_CORE : (i + 1) * NODES_PER_CORE]
            block[nz] = seg

    if spill_ids:
        sp = np.concatenate(spill_ids)
        np.add.at(full, r[sp], edges[sp])

    return full


# revision 10
# speedup vs baseline: 1.3665x; 1.0952x over previous
"""Trainium2 Bass kernel: segment_sum of edge features into nodes (GNN aggregation).

out[n, :] = sum of edges[e, :] over edges with receivers[e] == n, for
n in [0, 100000), edges [1000000, 64] fp32 — distributed over 8 NeuronCores.
Cores are value-sharded by receiver range (12500 nodes each, disjoint), so no
cross-core reduction is needed; the host concatenates the shards.

Device algorithm (degree-slotted static-ones matmul):
  - Host sorts each core's edges by receiver and packs them into "node-rows"
    of 4/2/1 slots (three regions by degree remainder, minimizing padding);
    a chunk = 128 slots = one TensorEngine matmul.
  - The stationary operand is a STATIC block-ones matrix (e.g. [128, 32] with
    ones[s, j] = 1 iff s//4 == j): out row j = sum of row j's slots. There is
    no per-chunk weight generation at all (no one-hot; VectorEngine is idle).
  - Edge fp32 values ride as fp16 hi + fp16 lo halves in one matmul: the
    output access pattern wraps both 64-column halves onto the same PSUM
    columns, and PSUM's per-element has_written accumulate folds hi+lo in
    hardware (end-to-end error ~2e-7 relative).
  - Column tiling (tile_position=(0, 32b)) packs 4 chunks per 128-partition
    PSUM block; 7 blocks share one PSUM bank; a single ScalarEngine copy
    flushes the bank and a contiguous DMA writes the rows out. Input DMAs run
    on the Sync-engine HWDGE ring, output DMAs on the Scalar-engine ring so
    the two streams don't serialize on one FIFO.
  - Host folds the ~3 rows per node with np.add.reduceat (region S4) and
    vectorized adds (S2/S1), then fixes up any capacity-spilled edges.
"""

import os

import numpy as np

N_EDGES = 1_000_000
N_NODES = 100_000
N_FEAT = 64
N_CORES = 8
NODES_PER_CORE = N_NODES // N_CORES  # 12500
BLK_W = 7

N4_BLOCKS = 245  # rows of 4 slots: cap 31360 (mean ~30500)
N2_BLOCKS = 28   # rows of 2 slots: cap 3584 (mean ~3125)
N1_BLOCKS = 28   # rows of 1 slot:  cap 3584 (mean ~3125)
N_BLOCKS = N4_BLOCKS + N2_BLOCKS + N1_BLOCKS  # 301
R4_CAP = N4_BLOCKS * 128
R2_CAP = N2_BLOCKS * 128
R1_CAP = N1_BLOCKS * 128
C4 = N4_BLOCKS * 4  # chunks in S4 region
C2 = N2_BLOCKS * 2
C1 = N1_BLOCKS * 1
C_CHUNKS = C4 + C2 + C1  # 1064

_NC_CACHE = None
LAST_RESULT = None


def _build_nc():
    global _NC_CACHE
    if _NC_CACHE is not None:
        return _NC_CACHE

    import concourse.bass as bass
    import concourse.tile as tile
    from concourse import bacc, mybir

    F16 = mybir.dt.float16
    F32 = mybir.dt.float32

    nc = bacc.Bacc("TRN2", target_bir_lowering=False)
    tokens = nc.dram_tensor("tokens", [128, C_CHUNKS, 128], F16, kind="ExternalInput")
    ones4 = nc.dram_tensor("ones4", [128, 32], F16, kind="ExternalInput")
    ones2 = nc.dram_tensor("ones2", [128, 64], F16, kind="ExternalInput")
    ones1 = nc.dram_tensor("ones1", [128, 128], F16, kind="ExternalInput")
    out = nc.dram_tensor("out", [128, N_BLOCKS, 64], F32, kind="ExternalOutput")

    with tile.TileContext(nc) as tc:
        with (
            tc.tile_pool(name="const", bufs=1) as const,
            tc.tile_pool(name="tok", bufs=6) as tokp,
            tc.tile_pool(name="ps", bufs=3, space="PSUM") as psp,
            tc.tile_pool(name="stage", bufs=3) as stp,
        ):
            ones4_t = const.tile([128, 32], F16)
            nc.sync.dma_start(ones4_t[:], ones4[:])
            ones2_t = const.tile([128, 64], F16)
            nc.sync.dma_start(ones2_t[:], ones2[:])
            ones1_t = const.tile([128, 128], F16)
            nc.sync.dma_start(ones1_t[:], ones1[:])

            def emit_mm(ps, blk, b, cols, ones_t, tok, lc):
                # out rows [cols*b, cols*(b+1)) of block blk, wrapped hi|lo
                pslice = ps[cols * b : cols * (b + 1),
                            blk * 64 : (blk + 1) * 64]
                o = bass.AP(pslice.tensor, pslice.offset,
                            [list(pslice.ap[0]), [0, 2], [1, 64]])
                nc.tensor.matmul(
                    out=o, lhsT=ones_t[:], rhs=tok[:, lc, :],
                    start=True, stop=True, tile_position=(0, cols * b))

            # region descriptors: (n_blocks, chunks_per_block, ones tile,
            #                      out-rows per chunk, chunk_base, block_base)
            # Emit the small S1/S2 regions first: their small token DMAs fill
            # the pipeline ramp; the uniform S4 stream then runs saturated.
            regions = [
                (N1_BLOCKS, 1, ones1_t, 128, C4 + C2, N4_BLOCKS + N2_BLOCKS),
                (N2_BLOCKS, 2, ones2_t, 64, C4, N4_BLOCKS),
                (N4_BLOCKS, 4, ones4_t, 32, 0, 0),
            ]
            for n_blocks, cpb, ones_t, cols, cbase, bbase in regions:
                group_sizes = [BLK_W] * (n_blocks // BLK_W)
                assert sum(group_sizes) == n_blocks
                b0 = 0
                for gw in group_sizes:
                    c0 = cbase + b0 * cpb
                    tok = tokp.tile([128, BLK_W * 4, 128], F16, tag="tok")
                    nc.sync.dma_start(
                        tok[:, 0 : gw * cpb, :], tokens[:, c0 : c0 + gw * cpb, :])
                    ps = psp.tile([128, BLK_W * 64], F32, tag="ps")
                    for blk in range(gw):
                        for b in range(cpb):
                            emit_mm(ps, blk, b, cols, ones_t, tok,
                                    blk * cpb + b)
                    stage = stp.tile([128, BLK_W * 64], F32, tag="stage")
                    nc.scalar.copy(stage[:, 0 : gw * 64], ps[:, 0 : gw * 64])
                    nc.scalar.dma_start(
                        out[:, bbase + b0 : bbase + b0 + gw, :],
                        stage[:, 0 : gw * 64])
                    b0 += gw
    nc.compile()
    _NC_CACHE = nc
    return nc


def _numpy_segment_sum(edges, receivers):
    out = np.zeros((N_NODES, N_FEAT), np.float32)
    r = np.asarray(receivers).astype(np.int64)
    ok = (r >= 0) & (r < N_NODES)
    np.add.at(out, r[ok], np.asarray(edges, np.float32)[ok])
    return out


def kernel(edges, nodes, receivers):
    global LAST_RESULT

    edges = np.ascontiguousarray(edges, dtype=np.float32)
    n_nodes = nodes.shape[0]
    r = np.asarray(receivers).astype(np.int64)
    if (
        edges.shape != (N_EDGES, N_FEAT)
        or n_nodes != N_NODES
        or r.shape != (N_EDGES,)
        or os.environ.get("KERNEL_FORCE_NUMPY")
    ):
        return _numpy_segment_sum(edges, receivers)

    order = np.argsort(r, kind="stable")
    r_s = r[order]
    bounds = np.searchsorted(r_s, NODES_PER_CORE * np.arange(N_CORES + 1))

    hi_all = edges.astype(np.float16)
    lo_all = (edges - hi_all.astype(np.float32)).astype(np.float16)

    ar = np.arange(128)
    ones4 = (ar[None, :] // 1 == 0).astype(np.float16)  # placeholder, fixed below
    ones4 = np.zeros((128, 32), np.float16)
    ones4[ar, ar // 4] = 1.0
    ones2 = np.zeros((128, 64), np.float16)
    ones2[ar, ar // 2] = 1.0
    ones1 = np.zeros((128, 128), np.float16)
    ones1[ar, ar] = 1.0

    in_maps = []
    spill_ids = []
    meta = []
    for i in range(N_CORES):
        lo_b, hi_b = bounds[i], bounds[i + 1]
        idx = order[lo_b:hi_b]
        rr = r_s[lo_b:hi_b] - NODES_PER_CORE * i

        d = np.bincount(rr, minlength=NODES_PER_CORE)
        rem = d & 3
        rows4_n = (d >> 2) + (rem == 3)
        rows2_n = (rem == 2).astype(np.int64)
        rows1_n = (rem == 1).astype(np.int64)
        if (
            rows4_n.sum() > R4_CAP
            or rows2_n.sum() > R2_CAP
            or rows1_n.sum() > R1_CAP
        ):
            cut = min(
                int(np.searchsorted(np.cumsum(rows4_n), R4_CAP, side="right")),
                int(np.searchsorted(np.cumsum(rows2_n), R2_CAP, side="right")),
                int(np.searchsorted(np.cumsum(rows1_n), R1_CAP, side="right")),
            )
            sp = rr >= cut
            spill_ids.append(idx[sp])
            idx, rr = idx[~sp], rr[~sp]
            d = np.bincount(rr, minlength=NODES_PER_CORE)
            rem = d & 3
            rows4_n = (d >> 2) + (rem == 3)
            rows2_n = (rem == 2).astype(np.int64)
            rows1_n = (rem == 1).astype(np.int64)

        def excl_cumsum(a):
            s = np.zeros_like(a)
            np.cumsum(a[:-1], out=s[1:])
            return s

        rs4 = excl_cumsum(rows4_n)
        rs2 = excl_cumsum(rows2_n)
        rs1 = excl_cumsum(rows1_n)
        node_first = excl_cumsum(d)

        rank = np.arange(len(rr)) - node_first[rr]
        e_rem = rem[rr]
        n_s4_edges = np.where(e_rem == 3, d[rr], (d[rr] >> 2) << 2)
        m4 = rank < n_s4_edges
        m2 = (~m4) & (e_rem == 2)
        m1 = (~m4) & (e_rem == 1)

        tokens = np.zeros((128, C_CHUNKS, 128), np.float16)

        row4 = rs4[rr[m4]] + (rank[m4] >> 2)
        lc = row4 >> 5
        p = (row4 & 31) * 4 + (rank[m4] & 3)
        tokens[p, lc, 0:64] = hi_all[idx[m4]]
        tokens[p, lc, 64:128] = lo_all[idx[m4]]

        row2 = rs2[rr[m2]]
        slot2 = rank[m2] - n_s4_edges[m2]
        lc = C4 + (row2 >> 6)
        p = (row2 & 63) * 2 + slot2
        tokens[p, lc, 0:64] = hi_all[idx[m2]]
        tokens[p, lc, 64:128] = lo_all[idx[m2]]

        row1 = rs1[rr[m1]]
        lc = C4 + C2 + (row1 >> 7)
        p = row1 & 127
        tokens[p, lc, 0:64] = hi_all[idx[m1]]
        tokens[p, lc, 64:128] = lo_all[idx[m1]]

        in_maps.append(
            {"tokens": tokens, "ones4": ones4, "ones2": ones2, "ones1": ones1}
        )
        meta.append((rows4_n, rs4, rows2_n, rs2, rows1_n, rs1))

    from concourse.bass_utils import run_bass_kernel_spmd

    nc = _build_nc()
    res = run_bass_kernel_spmd(nc, in_maps, core_ids=list(range(N_CORES)))
    LAST_RESULT = res

    full = np.zeros((N_NODES, N_FEAT), np.float32)
    for i in range(N_CORES):
        dev = res.results[i]["out"]  # [128, N_BLOCKS, 64]
        rows4_n, rs4, rows2_n, rs2, rows1_n, rs1 = meta[i]
        block = full[i * NODES_PER_CORE : (i + 1) * NODES_PER_CORE]

        arr4 = dev[:, 0:N4_BLOCKS, :].transpose(1, 0, 2).reshape(R4_CAP, 64)
        nz = rows4_n > 0
        if nz.any():
            block[nz] = np.add.reduceat(arr4, rs4[nz])

        arr2 = (
            dev[:, N4_BLOCKS : N4_BLOCKS + N2_BLOCKS, :]
            .transpose(1, 0, 2)
            .reshape(R2_CAP, 64)
        )
        m2n = rows2_n > 0
        if m2n.any():
            block[m2n] += arr2[rs2[m2n]]

        arr1 = (
            dev[:, N4_BLOCKS + N2_BLOCKS :, :].transpose(1, 0, 2).reshape(R1_CAP, 64)
        )
        m1n = rows1_n > 0
        if m1n.any():
            block[m1n] += arr1[rs1[m1n]]

    if spill_ids:
        sp = np.concatenate(spill_ids)
        np.add.at(full, r[sp], edges[sp])

    return full
